# revision 22
# baseline (speedup 1.0000x reference)
"""Trainium2 Bass kernel v6 for the decoder LSTM (B=256, T=2048, HID=128, OUT=6).

v4/v5 sharded TIME across the 8 cores (burn-in trick: the LSTM state is
contractive, so a zero state 32 steps before a segment converges to ~2e-7).
Each core ran ONE 288-step chain; the per-step serial chain (PE matmuls ->
ACT sigmoid -> DVE cell update -> ACT tanh -> DVE h-mul) left every engine
half idle.

v7 runs THREE interleaved chains per core (24 ragged time-segments of
86/85/85 output steps + 4 burn-in rounds across 8 cores = 90 rounds of 3
steps; measured 506 us on trn2 at rel_err 9.7e-3, vs 4203 us for the
batch-parallel v3). While chain A
is in its DVE phase, chain B uses ACT, etc. Each chain's four gate
pre-activation quarters [2g|i|f|o] land in one 2-bank PSUM tile so a
single sigmoid instruction covers them (the 352-cycle ACT overhead is
paid once, not twice); chains alternate emission priority each round;
filler matmuls ride in the wait-for-h stalls to keep the PE HAM
activity window fed.
"""

import os
import sys

for _p in ("/opt/trn_rl_repo", "/root/.axon_site/_ro/trn_rl_repo"):
    if os.path.isdir(_p) and _p not in sys.path:
        sys.path.insert(0, _p)

import numpy as np

B, T, VOCAB, EMB, HID, OUT = 256, 2048, 7, 20, 128, 6
NCORES = 8
VK = VOCAB + 1          # vocab + identity pseudo-token for core 0 burn-in
BURN = int(os.environ.get("V6_BURN", "4"))
NCH = 3                 # chains per core (24 time-segments across 8 cores)
SEG = T // NCORES       # 256 output steps per core
SEG3 = (86, 85, 85)     # ragged chain lengths (2048/24 is not integer)
CB = (0, 86, 171)       # chain base offset within the core's 256 steps
TTC = tuple(s + BURN for s in SEG3)   # rounds per chain (94, 93, 93)
TT = max(TTC)           # 94 rounds
FCW = 14                # rounds per fc PSUM window ([128, 6*14*6] f32)
NWIN = 7                # ceil(86/14) windows (last window partial)
OG = 4                  # rounds per one-hot DMA chunk
NFILL = int(os.environ.get("V6_FILL", "17"))
GI, GF, GG, GO = 0, 1, 2, 3  # PyTorch gate order in W_hh rows / table cols


def _split_overloaded_waits(nc, mybir, max_other=1):
    """walrus in this env rejects instructions with more than a couple of sem
    waits (and InstDrain with any). Move excess waits onto same-engine NoOps
    emitted just before; same-engine program order preserves semantics."""
    n_split = 0
    for f in nc.m.functions:
        for blk in f.blocks:
            out = []
            changed = False
            for inst in blk.instructions:
                si = inst.sync_info
                waits = list(si.on_wait) if si is not None and si.on_wait else []
                limit = 0 if isinstance(inst, mybir.InstDrain) else max_other
                if len(waits) > limit:
                    moved = waits if limit == 0 else waits[limit:]
                    keep = [] if limit == 0 else waits[:limit]
                    for i0, w in enumerate(moved):
                        nop = mybir.InstNoOp(
                            name=f"{inst.name}-wsplit{i0}", ins=[], outs=[]
                        )
                        nop.engine = inst.engine
                        nop.sync_info = mybir.SyncInfo(on_wait=[w], on_update=[])
                        out.append(nop)
                        n_split += 1
                    inst.sync_info = mybir.SyncInfo(
                        on_wait=keep,
                        on_update=list(si.on_update) if si.on_update else [],
                    )
                    changed = True
                out.append(inst)
            if changed:
                blk.instructions = out
    return n_split


def _patch_tile_drain():
    import concourse.tile as tile
    from concourse.vector_clock import ScopedClock, VectorClock

    def _drain_and_barrier_split(self, tick_clock, wait_clock):
        gc = tick_clock.global_clock
        n = len(gc)
        for j in range(n):
            if gc[j] <= 0:
                continue
            vec = [0] * n
            vec[j] = gc[j]
            nop = self.nc.sync.nop(nofuse=True, hint=f"drain_split_{j}")
            wait_clock.add_sem_waits(nop.ins, ScopedClock({None: VectorClock(vec)}))
        self.nc.sync.drain()
        self.nc.all_engine_barrier()
        assert self.sems is not None
        popped = self.nc._tile_sem_poison_stack.pop()
        assert popped is self._sem_poison
        self.nc.clear_and_free_semaphores(list(self.sems.allocated().values()))
        self.nc.all_engine_barrier()

    tile.TileContext._drain_and_barrier = _drain_and_barrier_split


_BUILD_CACHE = {}


def _build_nc(for_sim=False):
    key = (TT, for_sim, NFILL)
    if key in _BUILD_CACHE:
        return _BUILD_CACHE[key]
    import concourse.bass as bass
    import concourse.mybir as mybir
    import concourse.tile as tile

    _patch_tile_drain()

    f32 = mybir.dt.float32
    bf16 = mybir.dt.bfloat16
    AF = mybir.ActivationFunctionType
    ALU = mybir.AluOpType

    nc = bass.Bass("TRN2", target_bir_lowering=False, debug=False)
    TTP = ((TT + OG - 1) // OG) * OG  # one-hot rounds padded to whole chunks
    d_oh = nc.dram_tensor("onehot", [VK, TTP * NCH * B], bf16, kind="ExternalInput")
    d_c0 = nc.dram_tensor("c0T", [HID, NCH * B], bf16, kind="ExternalInput")
    d_w = nc.dram_tensor("w", [HID, 4 * HID], bf16, kind="ExternalInput")
    d_tbl = nc.dram_tensor("tbl", [VK, 4 * HID], bf16, kind="ExternalInput")
    d_wfc = nc.dram_tensor("wfc", [HID, OUT], bf16, kind="ExternalInput")
    d_bfcw = nc.dram_tensor("bfcw", [1, 2 * NCH * FCW * OUT], bf16,
                            kind="ExternalInput")
    CDIM = NWIN * NCH * 2 * FCW  # 588 logit rows of OUT
    d_out = nc.dram_tensor("out", [HID, CDIM, OUT], f32, kind="ExternalOutput")

    with tile.TileContext(nc) as tc, tc.tile_pool(name="const", bufs=1) as constp:
        w_sb = constp.tile([HID, 4 * HID], bf16, name="w_sb")
        tbl_sb = constp.tile([VK, 4 * HID], bf16, name="tbl_sb")
        wfc_sb = constp.tile([HID, OUT], bf16, name="wfc_sb")
        bfcw_sb = constp.tile([1, 2 * NCH * FCW * OUT], bf16, name="bfcw_sb")
        ones_sb = constp.tile([1, HID], bf16, name="ones_sb")
        cst2 = constp.tile([HID, NCH * B], bf16, name="cst2")
        h0_sb = constp.tile([HID, B], bf16, name="h0_sb")
        scr = constp.tile([HID, B], bf16, name="scr")
        fillsrc = constp.tile([HID, 2 * B], bf16, name="fillsrc")
        logit_sb = constp.tile([HID, CDIM * OUT], f32, name="logit_sb")
        den_sb = constp.tile([HID, CDIM], f32, name="den_sb")

        nc.sync.dma_start(w_sb[:], d_w.ap())
        nc.sync.dma_start(tbl_sb[:], d_tbl.ap())
        nc.sync.dma_start(wfc_sb[:], d_wfc.ap())
        nc.sync.dma_start(bfcw_sb[:], d_bfcw.ap())
        nc.sync.dma_start(cst2[:], d_c0.ap())
        nc.vector.memset(h0_sb[:], 0.0)
        nc.vector.memset(ones_sb[:], 1.0)
        nc.vector.memset(fillsrc[:], 0.0)
        nc.vector.memset(logit_sb[:], 0.0)
        # Pin the sigmoid_and_others table (contains tanh too) before the loop.
        nc.scalar.activation(scr[:], h0_sb[:], AF.Sigmoid)

        cst = [cst2[:, c * B : (c + 1) * B] for c in range(NCH)]

        with (
            tc.tile_pool(name="ohp", bufs=3) as ohp,
            tc.tile_pool(name="gatep", bufs=1, space="PSUM") as gatep,
            tc.tile_pool(name="fcp", bufs=1, space="PSUM") as fcp,
            tc.tile_pool(name="fillp", bufs=1, space="PSUM") as fillp,
            tc.tile_pool(name="ringp", bufs=3) as ringp,
            tc.tile_pool(name="workp", bufs=2) as workp,
        ):
            oh_tiles = [None] * (TT // OG + 1)

            def fetch_oh(chunk):
                ohc = ohp.tile([VK, OG * NCH * B], bf16, tag="oh")
                nc.sync.dma_start(
                    ohc[:],
                    d_oh.ap()[:, chunk * OG * NCH * B : (chunk + 1) * OG * NCH * B],
                )
                oh_tiles[chunk] = ohc

            fetch_oh(0)

            mergesig = os.environ.get("V6_MERGESIG", "1") == "1"

            def alloc_pair(r, c):
                """Allocate chain c's round-r gate PSUM (pair of banks, or one
                2-bank tile in mergesig mode) and prefill from the one-hot
                block (4 matmuls, K=VK, N=256). Quarter order [2g|i|f|o]."""
                if mergesig:
                    psAB = gatep.tile([128, 4 * B], f32, tag=f"psAB{c}")
                    quarters = [psAB[:, j * B : (j + 1) * B] for j in range(4)]
                else:
                    psA = gatep.tile([128, 2 * B], f32, tag=f"psA{c}")
                    psB = gatep.tile([128, 2 * B], f32, tag=f"psB{c}")
                    quarters = [
                        psA[:, 0:B], psA[:, B : 2 * B],
                        psB[:, 0:B], psB[:, B : 2 * B],
                    ]
                oh = oh_tiles[r // OG]
                col = ((r % OG) * NCH + c) * B
                for j, q in enumerate((GG, GI, GF, GO)):
                    nc.tensor.matmul(
                        quarters[j],
                        tbl_sb[:, q * HID : (q + 1) * HID],
                        oh[:, col : col + B],
                        start=(j % 2 == 0),
                        stop=False,
                        skip_group_check=True,
                    )
                if mergesig:
                    return (psAB, quarters)
                return (psA, psB, quarters)

            fill_ps = fillp.tile([128, 2 * B], f32, name="fill_ps")
            cur = [alloc_pair(0, c) for c in range(NCH)]
            nxt = [None] * NCH
            fcw_box = [None]
            pending_fc = [None] * NCH
            h_prev = [h0_sb[:]] * NCH

            def fc_round(t):
                """fc for output step t, every chain with a pending h and a
                valid step. Shared window bank [128, (chain, half, FCW, OUT)]
                f32; window NWIN-1 is partial and evacuated post-loop."""
                w0 = t % FCW
                if w0 == 0:
                    fcw_box[0] = fcp.tile(
                        [HID, 2 * NCH * FCW * OUT], f32, tag="fcw", name="fcw"
                    )
                    nc.tensor.matmul(
                        fcw_box[0][:], ones_sb[:], bfcw_sb[:],
                        start=True, stop=False, skip_group_check=True,
                    )
                fcw = fcw_box[0]
                live = [c for c in range(NCH)
                        if pending_fc[c] is not None and t < SEG3[c]]
                for c in live:
                    for hf in range(2):
                        o0 = (((c * 2) + hf) * FCW + w0) * OUT
                        nc.tensor.matmul(
                            fcw[:, o0 : o0 + OUT],
                            pending_fc[c][:, hf * HID : (hf + 1) * HID],
                            wfc_sb[:],
                            start=False,
                            stop=(hf == 1 and c == live[-1]
                                  and (w0 == FCW - 1 or t == SEG3[0] - 1)),
                            skip_group_check=True,
                        )
                    pending_fc[c] = None
                if w0 == FCW - 1 or t == SEG3[0] - 1:
                    win = t // FCW
                    W = 2 * NCH * FCW * OUT
                    nc.vector.tensor_copy(
                        logit_sb[:, win * W : (win + 1) * W], fcw[:]
                    )

            gpm = os.environ.get("V7_GPM", "0") == "1"
            ROT = ((0, 1, 2), (1, 2, 0), (2, 0, 1))
            for r in range(TT):
                rot = [c for c in ROT[r % NCH] if r < TTC[c]]
                # --- PE: gate matmuls (critical), then fc
                for c in rot:
                    quarters = cur[c][-1]
                    for j, q in enumerate((GG, GI, GF, GO)):
                        nc.tensor.matmul(
                            quarters[j],
                            w_sb[:, q * HID : (q + 1) * HID],
                            h_prev[c], start=False, stop=(j % 2 == 1),
                            skip_group_check=True,
                        )
                if any(p is not None for p in pending_fc):
                    fc_round(r - 1 - BURN)
                # --- ACT sigmoids + DVE cell updates, staggered so queues
                # drain in readiness order
                sgi = [None] * NCH
                sfo = [None] * NCH
                tg = [None] * NCH
                ig = [None] * NCH
                mmb = [None] * NCH
                tcl = [None] * NCH
                hsl = [None] * NCH

                def act_sgi(c):
                    sgi[c] = workp.tile(
                        [HID, 4 * B], bf16, tag=f"sgi{c}", name=f"sgi{c}"
                    )
                    nc.scalar.activation(sgi[c][:], cur[c][0][:], AF.Sigmoid)
                    sfo[c] = sgi[c]

                def dve_head(c):
                    tg[c] = workp.tile([HID, B], bf16, tag=f"tg{c}", name=f"tg{c}")
                    ig[c] = workp.tile([HID, B], bf16, tag=f"ig{c}", name=f"ig{c}")
                    nc.vector.tensor_scalar(
                        tg[c][:], sgi[c][:, 0:B], 2.0, 1.0,
                        op0=ALU.mult, op1=ALU.subtract,
                    )
                    nc.vector.tensor_mul(ig[c][:], tg[c][:], sgi[c][:, B : 2 * B])

                def dve_cell(c):
                    mmb[c] = workp.tile([HID, B], bf16, tag=f"mm{c}", name=f"mm{c}")
                    eng = nc.gpsimd if gpm else nc.vector
                    eng.tensor_mul(
                        mmb[c][:], sfo[c][:, 2 * B : 3 * B], cst[c]
                    )
                    nc.vector.tensor_add(cst[c], mmb[c][:], ig[c][:])

                def act_tanh(c):
                    tcl[c] = workp.tile([HID, B], bf16, tag=f"tcl{c}", name=f"tcl{c}")
                    nc.scalar.activation(tcl[c][:], cst[c], AF.Tanh)

                def dve_h(c):
                    hsl[c] = ringp.tile([HID, B], bf16, tag=f"h{c}", name=f"h{c}")
                    nc.vector.tensor_mul(
                        hsl[c][:], sfo[c][:, 3 * B : 4 * B], tcl[c][:]
                    )

                if len(rot) == NCH:
                    a, b, d = rot
                    act_sgi(a)
                    act_sgi(b)
                    dve_head(a)
                    dve_cell(a)
                    act_tanh(a)
                    act_sgi(d)
                    dve_head(b)
                    dve_h(a)
                    dve_cell(b)
                    act_tanh(b)
                    dve_head(d)
                    dve_h(b)
                    dve_cell(d)
                    act_tanh(d)
                    dve_h(d)
                else:
                    for c in rot:
                        act_sgi(c)
                        dve_head(c)
                        dve_cell(c)
                        act_tanh(c)
                        dve_h(c)

                # --- PE: prefill round r+1 (after the sigmoids that free the
                # recycled banks are emitted — bufs=1 WAR ordering)
                if (r + 1) % OG == 0 and (r + 1) // OG < (TT + OG - 1) // OG:
                    fetch_oh((r + 1) // OG)
                for c in rot:
                    if r + 1 < TTC[c]:
                        nxt[c] = alloc_pair(r + 1, c)
                # fillers ride in the wait-for-h stall before the next gates
                for _ in range(NFILL):
                    nc.tensor.matmul(
                        fill_ps[:, 0:B], w_sb[:, 0:HID], scr[:],
                        start=True, stop=True, skip_group_check=True,
                    )

                for c in rot:
                    h_prev[c] = hsl[c][:]
                    if r >= BURN and (r - BURN) < SEG3[c]:
                        pending_fc[c] = hsl[c][:]
                    if r + 1 < TTC[c]:
                        cur[c] = nxt[c]
            fc_round(SEG3[0] - 1)

        # ---- phase 2: softmax over OUT, windowed; layout is
        # [128, (win, chain, half, w, OUT)] and the host fixes the order. ----
        p3 = logit_sb[:].rearrange("p (c o) -> p c o", o=OUT)
        nc.scalar.activation(logit_sb[:], logit_sb[:], AF.Exp)
        NW = 2
        q = CDIM // NW
        for k in range(NW):
            lo, hi = k * q, (k + 1) * q
            nc.vector.reduce_sum(
                den_sb[:, lo:hi], p3[:, lo:hi, :], axis=mybir.AxisListType.X
            )
            nc.vector.reciprocal(den_sb[:, lo:hi], den_sb[:, lo:hi])
            rec_b = den_sb[:, lo:hi].unsqueeze(2).broadcast_to([HID, q, OUT])
            nc.vector.tensor_mul(p3[:, lo:hi, :], p3[:, lo:hi, :], rec_b)
            nc.sync.dma_start(d_out.ap()[:, lo:hi, :], p3[:, lo:hi, :])

    if not for_sim:
        _split_overloaded_waits(nc, mybir)
    _BUILD_CACHE[key] = nc
    return nc


def _host_prep(inputs, c0, W_ih, W_hh, b_ih, b_hh, W_fc, b_fc, emb):
    import ml_dtypes

    bf16 = ml_dtypes.bfloat16
    inputs = np.asarray(inputs)
    table_f = np.zeros((VK, 4 * HID), np.float32)
    table_f[:VOCAB] = emb @ W_ih.T + (b_ih + b_hh)
    table_f[VOCAB, GI * HID : (GI + 1) * HID] = -30.0
    table_f[VOCAB, GF * HID : (GF + 1) * HID] = 30.0
    table_f[VOCAB, GO * HID : (GO + 1) * HID] = -30.0
    w_f = W_hh.T.copy()
    table_f[:, GG * HID : (GG + 1) * HID] *= 2.0
    w_f[:, GG * HID : (GG + 1) * HID] *= 2.0
    table = table_f.astype(bf16)
    w = np.ascontiguousarray(w_f.astype(bf16))
    wfc = np.ascontiguousarray(W_fc.T.astype(bf16))
    bfcw = np.ascontiguousarray(
        np.tile(b_fc.astype(bf16), 2 * NCH * FCW).reshape(1, 2 * NCH * FCW * OUT)
    )
    c0T = np.ascontiguousarray(c0[0].T.astype(bf16))
    TTP = ((TT + OG - 1) // OG) * OG
    in_maps = []
    for core in range(NCORES):
        tok_all = np.zeros((TTP, NCH, B), np.int64)
        for c in range(NCH):
            st = core * SEG + CB[c]
            if core == 0 and c == 0:
                tok = np.concatenate(
                    [np.full((B, BURN), VOCAB, np.int64),
                     inputs[:, : SEG3[c]]], axis=1,
                )
            else:
                tok = inputs[:, st - BURN : st + SEG3[c]]
            tok_all[: TTC[c], c, :] = tok.T
        oh = np.zeros((VK, TTP * NCH * B), dtype=bf16)
        cols = np.arange(TTP * NCH * B)
        oh[tok_all.reshape(-1), cols] = 1.0
        cc = np.zeros((HID, NCH * B), dtype=bf16)
        if core == 0:
            cc[:, 0:B] = c0T
        in_maps.append(
            {
                "onehot": oh,
                "c0T": cc,
                "w": w,
                "tbl": table,
                "wfc": wfc,
                "bfcw": bfcw,
            }
        )
    return in_maps


def _run(inputs, c0, W_ih, W_hh, b_ih, b_hh, W_fc, b_fc, emb, trace=False):
    from concourse.bass_utils import run_bass_kernel_spmd

    nc = _build_nc()
    in_maps = _host_prep(inputs, c0, W_ih, W_hh, b_ih, b_hh, W_fc, b_fc, emb)
    res = run_bass_kernel_spmd(
        nc, in_maps, core_ids=list(range(NCORES)), trace=trace
    )
    segs = []
    for core in range(NCORES):
        arr = res.results[core]["out"]  # [128, NWIN*NCH*2*FCW, 6]
        arr = arr.reshape(HID, NWIN, NCH, 2, FCW, OUT)
        out_c = np.empty((B, SEG, OUT), np.float32)
        for c in range(NCH):
            for win in range(NWIN):
                wv = min(FCW, SEG3[c] - win * FCW)
                if wv <= 0:
                    break
                blk = arr[:, win, c, :, :wv, :]        # [128, 2, wv, 6]
                blk = blk.transpose(1, 0, 2, 3).reshape(B, wv, OUT)
                t0c = CB[c] + win * FCW
                out_c[:, t0c : t0c + wv] = blk
        segs.append(out_c)
    out = np.concatenate(segs, axis=1)
    return out, res


def kernel(inputs, c0, W_ih, W_hh, b_ih, b_hh, W_fc, b_fc, emb):
    out, _ = _run(
        np.asarray(inputs), np.asarray(c0), np.asarray(W_ih), np.asarray(W_hh),
        np.asarray(b_ih), np.asarray(b_hh), np.asarray(W_fc), np.asarray(b_fc),
        np.asarray(emb),
    )
    return out


# revision 23
# speedup vs baseline: 1.0076x; 1.0076x over previous
"""Trainium2 Bass kernel v6 for the decoder LSTM (B=256, T=2048, HID=128, OUT=6).

v4/v5 sharded TIME across the 8 cores (burn-in trick: the LSTM state is
contractive, so a zero state 32 steps before a segment converges to ~2e-7).
Each core ran ONE 288-step chain; the per-step serial chain (PE matmuls ->
ACT sigmoid -> DVE cell update -> ACT tanh -> DVE h-mul) left every engine
half idle.

v7 runs THREE interleaved chains per core (24 ragged time-segments of
86/85/85 output steps + 4 burn-in rounds across 8 cores = 90 rounds of 3
steps; measured 506 us on trn2 at rel_err 9.7e-3, vs 4203 us for the
batch-parallel v3). While chain A
is in its DVE phase, chain B uses ACT, etc. Each chain's four gate
pre-activation quarters [2g|i|f|o] land in one 2-bank PSUM tile so a
single sigmoid instruction covers them (the 352-cycle ACT overhead is
paid once, not twice); chains alternate emission priority each round;
filler matmuls ride in the wait-for-h stalls to keep the PE HAM
activity window fed.
"""

import os
import sys

for _p in ("/opt/trn_rl_repo", "/root/.axon_site/_ro/trn_rl_repo"):
    if os.path.isdir(_p) and _p not in sys.path:
        sys.path.insert(0, _p)

import numpy as np

B, T, VOCAB, EMB, HID, OUT = 256, 2048, 7, 20, 128, 6
NCORES = 8
VK = VOCAB + 1          # vocab + identity pseudo-token for core 0 burn-in
BURN = int(os.environ.get("V6_BURN", "4"))
NCH = 3                 # chains per core (24 time-segments across 8 cores)
SEG = T // NCORES       # 256 output steps per core
SEG3 = (86, 85, 85)     # ragged chain lengths (2048/24 is not integer)
CB = (0, 86, 171)       # chain base offset within the core's 256 steps
TTC = tuple(s + BURN for s in SEG3)   # rounds per chain (94, 93, 93)
TT = max(TTC)           # 94 rounds
FCW = 14                # rounds per fc PSUM window ([128, 6*14*6] f32)
NWIN = 7                # ceil(86/14) windows (last window partial)
OG = 4                  # rounds per one-hot DMA chunk
NFILL = int(os.environ.get("V6_FILL", "17"))
GI, GF, GG, GO = 0, 1, 2, 3  # PyTorch gate order in W_hh rows / table cols


def _split_overloaded_waits(nc, mybir, max_other=1):
    """walrus in this env rejects instructions with more than a couple of sem
    waits (and InstDrain with any). Move excess waits onto same-engine NoOps
    emitted just before; same-engine program order preserves semantics."""
    n_split = 0
    for f in nc.m.functions:
        for blk in f.blocks:
            out = []
            changed = False
            for inst in blk.instructions:
                si = inst.sync_info
                waits = list(si.on_wait) if si is not None and si.on_wait else []
                limit = 0 if isinstance(inst, mybir.InstDrain) else max_other
                if len(waits) > limit:
                    moved = waits if limit == 0 else waits[limit:]
                    keep = [] if limit == 0 else waits[:limit]
                    for i0, w in enumerate(moved):
                        nop = mybir.InstNoOp(
                            name=f"{inst.name}-wsplit{i0}", ins=[], outs=[]
                        )
                        nop.engine = inst.engine
                        nop.sync_info = mybir.SyncInfo(on_wait=[w], on_update=[])
                        out.append(nop)
                        n_split += 1
                    inst.sync_info = mybir.SyncInfo(
                        on_wait=keep,
                        on_update=list(si.on_update) if si.on_update else [],
                    )
                    changed = True
                out.append(inst)
            if changed:
                blk.instructions = out
    return n_split


def _patch_tile_drain():
    import concourse.tile as tile
    from concourse.vector_clock import ScopedClock, VectorClock

    def _drain_and_barrier_split(self, tick_clock, wait_clock):
        gc = tick_clock.global_clock
        n = len(gc)
        for j in range(n):
            if gc[j] <= 0:
                continue
            vec = [0] * n
            vec[j] = gc[j]
            nop = self.nc.sync.nop(nofuse=True, hint=f"drain_split_{j}")
            wait_clock.add_sem_waits(nop.ins, ScopedClock({None: VectorClock(vec)}))
        self.nc.sync.drain()
        self.nc.all_engine_barrier()
        assert self.sems is not None
        popped = self.nc._tile_sem_poison_stack.pop()
        assert popped is self._sem_poison
        self.nc.clear_and_free_semaphores(list(self.sems.allocated().values()))
        self.nc.all_engine_barrier()

    tile.TileContext._drain_and_barrier = _drain_and_barrier_split


_BUILD_CACHE = {}


def _build_nc(for_sim=False):
    key = (TT, for_sim, NFILL)
    if key in _BUILD_CACHE:
        return _BUILD_CACHE[key]
    import concourse.bass as bass
    import concourse.mybir as mybir
    import concourse.tile as tile

    _patch_tile_drain()

    f32 = mybir.dt.float32
    bf16 = mybir.dt.bfloat16
    AF = mybir.ActivationFunctionType
    ALU = mybir.AluOpType

    nc = bass.Bass("TRN2", target_bir_lowering=False, debug=False)
    TTP = ((TT + OG - 1) // OG) * OG  # one-hot rounds padded to whole chunks
    d_oh = nc.dram_tensor("onehot", [VK, TTP * NCH * B], bf16, kind="ExternalInput")
    d_c0 = nc.dram_tensor("c0T", [HID, NCH * B], bf16, kind="ExternalInput")
    d_w = nc.dram_tensor("w", [HID, 4 * HID], bf16, kind="ExternalInput")
    d_tbl = nc.dram_tensor("tbl", [VK, 4 * HID], bf16, kind="ExternalInput")
    d_wfc = nc.dram_tensor("wfc", [HID, OUT], bf16, kind="ExternalInput")
    d_bfcw = nc.dram_tensor("bfcw", [1, 2 * NCH * FCW * OUT], bf16,
                            kind="ExternalInput")
    CDIM = NWIN * NCH * 2 * FCW  # 588 logit rows of OUT
    d_out = nc.dram_tensor("out", [HID, CDIM, OUT], f32, kind="ExternalOutput")

    with tile.TileContext(nc) as tc, tc.tile_pool(name="const", bufs=1) as constp:
        w_sb = constp.tile([HID, 4 * HID], bf16, name="w_sb")
        tbl_sb = constp.tile([VK, 4 * HID], bf16, name="tbl_sb")
        wfc_sb = constp.tile([HID, OUT], bf16, name="wfc_sb")
        bfcw_sb = constp.tile([1, 2 * NCH * FCW * OUT], bf16, name="bfcw_sb")
        ones_sb = constp.tile([1, HID], bf16, name="ones_sb")
        cst2 = constp.tile([HID, NCH * B], bf16, name="cst2")
        h0_sb = constp.tile([HID, B], bf16, name="h0_sb")
        scr = constp.tile([HID, B], bf16, name="scr")
        fillsrc = constp.tile([HID, 2 * B], bf16, name="fillsrc")
        logit_sb = constp.tile([HID, CDIM * OUT], f32, name="logit_sb")
        den_sb = constp.tile([HID, CDIM], f32, name="den_sb")

        nc.sync.dma_start(w_sb[:], d_w.ap())
        nc.sync.dma_start(tbl_sb[:], d_tbl.ap())
        nc.sync.dma_start(wfc_sb[:], d_wfc.ap())
        nc.sync.dma_start(bfcw_sb[:], d_bfcw.ap())
        nc.sync.dma_start(cst2[:], d_c0.ap())
        nc.vector.memset(h0_sb[:], 0.0)
        nc.vector.memset(ones_sb[:], 1.0)
        nc.vector.memset(fillsrc[:], 0.0)
        nc.vector.memset(logit_sb[:], 0.0)
        # Pin the sigmoid_and_others table (contains tanh too) before the loop.
        nc.scalar.activation(scr[:], h0_sb[:], AF.Sigmoid)

        cst = [cst2[:, c * B : (c + 1) * B] for c in range(NCH)]

        with (
            tc.tile_pool(name="ohp", bufs=3) as ohp,
            tc.tile_pool(name="gatep", bufs=1, space="PSUM") as gatep,
            tc.tile_pool(name="fcp", bufs=1, space="PSUM") as fcp,
            tc.tile_pool(name="fillp", bufs=1, space="PSUM") as fillp,
            tc.tile_pool(name="ringp", bufs=3) as ringp,
            tc.tile_pool(name="workp", bufs=2) as workp,
        ):
            oh_tiles = [None] * (TT // OG + 1)

            def fetch_oh(chunk):
                ohc = ohp.tile([VK, OG * NCH * B], bf16, tag="oh")
                nc.sync.dma_start(
                    ohc[:],
                    d_oh.ap()[:, chunk * OG * NCH * B : (chunk + 1) * OG * NCH * B],
                )
                oh_tiles[chunk] = ohc

            fetch_oh(0)

            mergesig = os.environ.get("V6_MERGESIG", "1") == "1"

            def alloc_pair(r, c):
                """Allocate chain c's round-r gate PSUM (pair of banks, or one
                2-bank tile in mergesig mode) and prefill from the one-hot
                block (4 matmuls, K=VK, N=256). Quarter order [2g|i|f|o]."""
                if mergesig:
                    psAB = gatep.tile([128, 4 * B], f32, tag=f"psAB{c}")
                    quarters = [psAB[:, j * B : (j + 1) * B] for j in range(4)]
                else:
                    psA = gatep.tile([128, 2 * B], f32, tag=f"psA{c}")
                    psB = gatep.tile([128, 2 * B], f32, tag=f"psB{c}")
                    quarters = [
                        psA[:, 0:B], psA[:, B : 2 * B],
                        psB[:, 0:B], psB[:, B : 2 * B],
                    ]
                oh = oh_tiles[r // OG]
                col = ((r % OG) * NCH + c) * B
                for j, q in enumerate((GG, GI, GF, GO)):
                    nc.tensor.matmul(
                        quarters[j],
                        tbl_sb[:, q * HID : (q + 1) * HID],
                        oh[:, col : col + B],
                        start=(j % 2 == 0),
                        stop=False,
                        skip_group_check=True,
                    )
                if mergesig:
                    return (psAB, quarters)
                return (psA, psB, quarters)

            fill_ps = fillp.tile([128, 2 * B], f32, name="fill_ps")
            cur = [alloc_pair(0, c) for c in range(NCH)]
            nxt = [None] * NCH
            fcw_box = [None]
            pending_fc = [None] * NCH
            h_prev = [h0_sb[:]] * NCH

            def fc_round(t):
                """fc for output step t, every chain with a pending h and a
                valid step. Shared window bank [128, (chain, half, FCW, OUT)]
                f32; window NWIN-1 is partial and evacuated post-loop."""
                w0 = t % FCW
                if w0 == 0:
                    fcw_box[0] = fcp.tile(
                        [HID, 2 * NCH * FCW * OUT], f32, tag="fcw", name="fcw"
                    )
                    nc.tensor.matmul(
                        fcw_box[0][:], ones_sb[:], bfcw_sb[:],
                        start=True, stop=False, skip_group_check=True,
                    )
                fcw = fcw_box[0]
                live = [c for c in range(NCH)
                        if pending_fc[c] is not None and t < SEG3[c]]
                for c in live:
                    for hf in range(2):
                        o0 = (((c * 2) + hf) * FCW + w0) * OUT
                        nc.tensor.matmul(
                            fcw[:, o0 : o0 + OUT],
                            pending_fc[c][:, hf * HID : (hf + 1) * HID],
                            wfc_sb[:],
                            start=False,
                            stop=(hf == 1 and c == live[-1]
                                  and (w0 == FCW - 1 or t == SEG3[0] - 1)),
                            skip_group_check=True,
                        )
                    pending_fc[c] = None
                if w0 == FCW - 1 or t == SEG3[0] - 1:
                    win = t // FCW
                    W = 2 * NCH * FCW * OUT
                    nc.scalar.copy(
                        logit_sb[:, win * W : (win + 1) * W], fcw[:]
                    )

            gpm = os.environ.get("V7_GPM", "0") == "1"
            ROT = ((0, 1, 2), (1, 2, 0), (2, 0, 1))
            for r in range(TT):
                rot = [c for c in ROT[r % NCH] if r < TTC[c]]
                # --- PE: gate matmuls (critical), then fc
                for c in rot:
                    quarters = cur[c][-1]
                    for j, q in enumerate((GG, GI, GF, GO)):
                        nc.tensor.matmul(
                            quarters[j],
                            w_sb[:, q * HID : (q + 1) * HID],
                            h_prev[c], start=False, stop=(j % 2 == 1),
                            skip_group_check=True,
                        )
                if any(p is not None for p in pending_fc):
                    fc_round(r - 1 - BURN)
                # --- ACT sigmoids + DVE cell updates, staggered so queues
                # drain in readiness order
                sgi = [None] * NCH
                sfo = [None] * NCH
                tg = [None] * NCH
                ig = [None] * NCH
                mmb = [None] * NCH
                tcl = [None] * NCH
                hsl = [None] * NCH

                def act_sgi(c):
                    sgi[c] = workp.tile(
                        [HID, 4 * B], bf16, tag=f"sgi{c}", name=f"sgi{c}"
                    )
                    nc.scalar.activation(sgi[c][:], cur[c][0][:], AF.Sigmoid)
                    sfo[c] = sgi[c]

                def dve_head(c):
                    tg[c] = workp.tile([HID, B], bf16, tag=f"tg{c}", name=f"tg{c}")
                    ig[c] = workp.tile([HID, B], bf16, tag=f"ig{c}", name=f"ig{c}")
                    nc.vector.tensor_scalar(
                        tg[c][:], sgi[c][:, 0:B], 2.0, 1.0,
                        op0=ALU.mult, op1=ALU.subtract,
                    )
                    nc.vector.tensor_mul(ig[c][:], tg[c][:], sgi[c][:, B : 2 * B])

                def dve_cell(c):
                    mmb[c] = workp.tile([HID, B], bf16, tag=f"mm{c}", name=f"mm{c}")
                    eng = nc.gpsimd if gpm else nc.vector
                    eng.tensor_mul(
                        mmb[c][:], sfo[c][:, 2 * B : 3 * B], cst[c]
                    )
                    nc.vector.tensor_add(cst[c], mmb[c][:], ig[c][:])

                def act_tanh(c):
                    tcl[c] = workp.tile([HID, B], bf16, tag=f"tcl{c}", name=f"tcl{c}")
                    nc.scalar.activation(tcl[c][:], cst[c], AF.Tanh)

                def dve_h(c):
                    hsl[c] = ringp.tile([HID, B], bf16, tag=f"h{c}", name=f"h{c}")
                    nc.vector.tensor_mul(
                        hsl[c][:], sfo[c][:, 3 * B : 4 * B], tcl[c][:]
                    )

                if len(rot) == NCH:
                    a, b, d = rot
                    act_sgi(a)
                    act_sgi(b)
                    dve_head(a)
                    dve_cell(a)
                    act_tanh(a)
                    act_sgi(d)
                    dve_head(b)
                    dve_h(a)
                    dve_cell(b)
                    act_tanh(b)
                    dve_head(d)
                    dve_h(b)
                    dve_cell(d)
                    act_tanh(d)
                    dve_h(d)
                else:
                    for c in rot:
                        act_sgi(c)
                        dve_head(c)
                        dve_cell(c)
                        act_tanh(c)
                        dve_h(c)

                # --- PE: prefill round r+1 (after the sigmoids that free the
                # recycled banks are emitted — bufs=1 WAR ordering)
                if (r + 1) % OG == 0 and (r + 1) // OG < (TT + OG - 1) // OG:
                    fetch_oh((r + 1) // OG)
                for c in rot:
                    if r + 1 < TTC[c]:
                        nxt[c] = alloc_pair(r + 1, c)
                # fillers ride in the wait-for-h stall before the next gates
                for _ in range(NFILL):
                    nc.tensor.matmul(
                        fill_ps[:, 0:B], w_sb[:, 0:HID], scr[:],
                        start=True, stop=True, skip_group_check=True,
                    )

                for c in rot:
                    h_prev[c] = hsl[c][:]
                    if r >= BURN and (r - BURN) < SEG3[c]:
                        pending_fc[c] = hsl[c][:]
                    if r + 1 < TTC[c]:
                        cur[c] = nxt[c]
            fc_round(SEG3[0] - 1)

        # ---- phase 2: softmax over OUT, windowed; layout is
        # [128, (win, chain, half, w, OUT)] and the host fixes the order. ----
        p3 = logit_sb[:].rearrange("p (c o) -> p c o", o=OUT)
        NW = 4
        q = CDIM // NW
        for k in range(NW):
            lo, hi = k * q, (k + 1) * q
            nc.scalar.activation(
                logit_sb[:, lo * OUT : hi * OUT],
                logit_sb[:, lo * OUT : hi * OUT], AF.Exp,
            )
            nc.vector.reduce_sum(
                den_sb[:, lo:hi], p3[:, lo:hi, :], axis=mybir.AxisListType.X
            )
            nc.vector.reciprocal(den_sb[:, lo:hi], den_sb[:, lo:hi])
            rec_b = den_sb[:, lo:hi].unsqueeze(2).broadcast_to([HID, q, OUT])
            nc.vector.tensor_mul(p3[:, lo:hi, :], p3[:, lo:hi, :], rec_b)
            nc.sync.dma_start(d_out.ap()[:, lo:hi, :], p3[:, lo:hi, :])

    if not for_sim:
        _split_overloaded_waits(nc, mybir)
    _BUILD_CACHE[key] = nc
    return nc


def _host_prep(inputs, c0, W_ih, W_hh, b_ih, b_hh, W_fc, b_fc, emb):
    import ml_dtypes

    bf16 = ml_dtypes.bfloat16
    inputs = np.asarray(inputs)
    table_f = np.zeros((VK, 4 * HID), np.float32)
    table_f[:VOCAB] = emb @ W_ih.T + (b_ih + b_hh)
    table_f[VOCAB, GI * HID : (GI + 1) * HID] = -30.0
    table_f[VOCAB, GF * HID : (GF + 1) * HID] = 30.0
    table_f[VOCAB, GO * HID : (GO + 1) * HID] = -30.0
    w_f = W_hh.T.copy()
    table_f[:, GG * HID : (GG + 1) * HID] *= 2.0
    w_f[:, GG * HID : (GG + 1) * HID] *= 2.0
    table = table_f.astype(bf16)
    w = np.ascontiguousarray(w_f.astype(bf16))
    wfc = np.ascontiguousarray(W_fc.T.astype(bf16))
    bfcw = np.ascontiguousarray(
        np.tile(b_fc.astype(bf16), 2 * NCH * FCW).reshape(1, 2 * NCH * FCW * OUT)
    )
    c0T = np.ascontiguousarray(c0[0].T.astype(bf16))
    TTP = ((TT + OG - 1) // OG) * OG
    in_maps = []
    for core in range(NCORES):
        tok_all = np.zeros((TTP, NCH, B), np.int64)
        for c in range(NCH):
            st = core * SEG + CB[c]
            if core == 0 and c == 0:
                tok = np.concatenate(
                    [np.full((B, BURN), VOCAB, np.int64),
                     inputs[:, : SEG3[c]]], axis=1,
                )
            else:
                tok = inputs[:, st - BURN : st + SEG3[c]]
            tok_all[: TTC[c], c, :] = tok.T
        oh = np.zeros((VK, TTP * NCH * B), dtype=bf16)
        cols = np.arange(TTP * NCH * B)
        oh[tok_all.reshape(-1), cols] = 1.0
        cc = np.zeros((HID, NCH * B), dtype=bf16)
        if core == 0:
            cc[:, 0:B] = c0T
        in_maps.append(
            {
                "onehot": oh,
                "c0T": cc,
                "w": w,
                "tbl": table,
                "wfc": wfc,
                "bfcw": bfcw,
            }
        )
    return in_maps


def _run(inputs, c0, W_ih, W_hh, b_ih, b_hh, W_fc, b_fc, emb, trace=False):
    from concourse.bass_utils import run_bass_kernel_spmd

    nc = _build_nc()
    in_maps = _host_prep(inputs, c0, W_ih, W_hh, b_ih, b_hh, W_fc, b_fc, emb)
    res = run_bass_kernel_spmd(
        nc, in_maps, core_ids=list(range(NCORES)), trace=trace
    )
    segs = []
    for core in range(NCORES):
        arr = res.results[core]["out"]  # [128, NWIN*NCH*2*FCW, 6]
        arr = arr.reshape(HID, NWIN, NCH, 2, FCW, OUT)
        out_c = np.empty((B, SEG, OUT), np.float32)
        for c in range(NCH):
            for win in range(NWIN):
                wv = min(FCW, SEG3[c] - win * FCW)
                if wv <= 0:
                    break
                blk = arr[:, win, c, :, :wv, :]        # [128, 2, wv, 6]
                blk = blk.transpose(1, 0, 2, 3).reshape(B, wv, OUT)
                t0c = CB[c] + win * FCW
                out_c[:, t0c : t0c + wv] = blk
        segs.append(out_c)
    out = np.concatenate(segs, axis=1)
    return out, res


def kernel(inputs, c0, W_ih, W_hh, b_ih, b_hh, W_fc, b_fc, emb):
    out, _ = _run(
        np.asarray(inputs), np.asarray(c0), np.asarray(W_ih), np.asarray(W_hh),
        np.asarray(b_ih), np.asarray(b_hh), np.asarray(W_fc), np.asarray(b_fc),
        np.asarray(emb),
    )
    return out


# revision 24
# speedup vs baseline: 1.1633x; 1.1545x over previous
"""Trainium2 Bass kernel v6 for the decoder LSTM (B=256, T=2048, HID=128, OUT=6).

v4/v5 sharded TIME across the 8 cores (burn-in trick: the LSTM state is
contractive, so a zero state 32 steps before a segment converges to ~2e-7).
Each core ran ONE 288-step chain; the per-step serial chain (PE matmuls ->
ACT sigmoid -> DVE cell update -> ACT tanh -> DVE h-mul) left every engine
half idle.

v7 runs THREE interleaved chains per core (24 ragged time-segments of
86/85/85 output steps + 4 burn-in rounds across 8 cores = 90 rounds of 3
steps; measured 506 us on trn2 at rel_err 9.7e-3, vs 4203 us for the
batch-parallel v3). While chain A
is in its DVE phase, chain B uses ACT, etc. Each chain's four gate
pre-activation quarters [2g|i|f|o] land in one 2-bank PSUM tile so a
single sigmoid instruction covers them (the 352-cycle ACT overhead is
paid once, not twice); chains alternate emission priority each round;
filler matmuls ride in the wait-for-h stalls to keep the PE HAM
activity window fed.
"""

import os
import sys

for _p in ("/opt/trn_rl_repo", "/root/.axon_site/_ro/trn_rl_repo"):
    if os.path.isdir(_p) and _p not in sys.path:
        sys.path.insert(0, _p)

import numpy as np

B, T, VOCAB, EMB, HID, OUT = 256, 2048, 7, 20, 128, 6
NCORES = 8
VK = VOCAB + 1          # vocab + identity pseudo-token for core 0 burn-in
BURN = int(os.environ.get("V6_BURN", "4"))
NCH = 3                 # chains per core (24 time-segments across 8 cores)
SEG = T // NCORES       # 256 output steps per core
SEG3 = (86, 85, 85)     # ragged chain lengths (2048/24 is not integer)
CB = (0, 86, 171)       # chain base offset within the core's 256 steps
TTC = tuple(s + BURN for s in SEG3)   # rounds per chain (94, 93, 93)
TT = max(TTC)           # 94 rounds
FCW = 14                # rounds per fc PSUM window ([128, 6*14*6] f32)
NWIN = 7                # ceil(86/14) windows (last window partial)
OG = 4                  # rounds per one-hot DMA chunk
NFILL = int(os.environ.get("V6_FILL", "17"))
GI, GF, GG, GO = 0, 1, 2, 3  # PyTorch gate order in W_hh rows / table cols


def _split_overloaded_waits(nc, mybir, max_other=1):
    """walrus in this env rejects instructions with more than a couple of sem
    waits (and InstDrain with any). Move excess waits onto same-engine NoOps
    emitted just before; same-engine program order preserves semantics."""
    n_split = 0
    for f in nc.m.functions:
        for blk in f.blocks:
            out = []
            changed = False
            for inst in blk.instructions:
                si = inst.sync_info
                waits = list(si.on_wait) if si is not None and si.on_wait else []
                limit = 0 if isinstance(inst, mybir.InstDrain) else max_other
                if len(waits) > limit:
                    moved = waits if limit == 0 else waits[limit:]
                    keep = [] if limit == 0 else waits[:limit]
                    for i0, w in enumerate(moved):
                        nop = mybir.InstNoOp(
                            name=f"{inst.name}-wsplit{i0}", ins=[], outs=[]
                        )
                        nop.engine = inst.engine
                        nop.sync_info = mybir.SyncInfo(on_wait=[w], on_update=[])
                        out.append(nop)
                        n_split += 1
                    inst.sync_info = mybir.SyncInfo(
                        on_wait=keep,
                        on_update=list(si.on_update) if si.on_update else [],
                    )
                    changed = True
                out.append(inst)
            if changed:
                blk.instructions = out
    return n_split


def _patch_tile_drain():
    import concourse.tile as tile
    from concourse.vector_clock import ScopedClock, VectorClock

    def _drain_and_barrier_split(self, tick_clock, wait_clock):
        gc = tick_clock.global_clock
        n = len(gc)
        for j in range(n):
            if gc[j] <= 0:
                continue
            vec = [0] * n
            vec[j] = gc[j]
            nop = self.nc.sync.nop(nofuse=True, hint=f"drain_split_{j}")
            wait_clock.add_sem_waits(nop.ins, ScopedClock({None: VectorClock(vec)}))
        self.nc.sync.drain()
        self.nc.all_engine_barrier()
        assert self.sems is not None
        popped = self.nc._tile_sem_poison_stack.pop()
        assert popped is self._sem_poison
        self.nc.clear_and_free_semaphores(list(self.sems.allocated().values()))
        self.nc.all_engine_barrier()

    tile.TileContext._drain_and_barrier = _drain_and_barrier_split


_BUILD_CACHE = {}


def _build_nc(for_sim=False):
    key = (TT, for_sim, NFILL)
    if key in _BUILD_CACHE:
        return _BUILD_CACHE[key]
    import concourse.bass as bass
    import concourse.mybir as mybir
    import concourse.tile as tile

    _patch_tile_drain()

    f32 = mybir.dt.float32
    bf16 = mybir.dt.bfloat16
    AF = mybir.ActivationFunctionType
    ALU = mybir.AluOpType

    nc = bass.Bass("TRN2", target_bir_lowering=False, debug=False)
    TTP = ((TT + OG - 1) // OG) * OG  # one-hot rounds padded to whole chunks
    d_oh = nc.dram_tensor("onehot", [VK, TTP * NCH * B], bf16, kind="ExternalInput")
    d_c0 = nc.dram_tensor("c0T", [HID, NCH * B], bf16, kind="ExternalInput")
    d_w = nc.dram_tensor("w", [HID, 4 * HID], bf16, kind="ExternalInput")
    d_tbl = nc.dram_tensor("tbl", [VK, 4 * HID], bf16, kind="ExternalInput")
    d_wfc = nc.dram_tensor("wfc", [HID, OUT], bf16, kind="ExternalInput")
    d_bfcw = nc.dram_tensor("bfcw", [1, 2 * NCH * FCW * OUT], bf16,
                            kind="ExternalInput")
    CDIM = NWIN * NCH * 2 * FCW  # 588 logit rows of OUT
    d_out = nc.dram_tensor("out", [HID, CDIM, OUT], f32, kind="ExternalOutput")

    with tile.TileContext(nc) as tc, tc.tile_pool(name="const", bufs=1) as constp:
        w_sb = constp.tile([HID, 4 * HID], bf16, name="w_sb")
        tbl_sb = constp.tile([VK, 4 * HID], bf16, name="tbl_sb")
        wfc_sb = constp.tile([HID, OUT], bf16, name="wfc_sb")
        bfcw_sb = constp.tile([1, 2 * NCH * FCW * OUT], bf16, name="bfcw_sb")
        ones_sb = constp.tile([1, HID], bf16, name="ones_sb")
        cst2 = constp.tile([HID, NCH * B], bf16, name="cst2")
        h0_sb = constp.tile([HID, B], bf16, name="h0_sb")
        scr = constp.tile([HID, B], bf16, name="scr")
        fillsrc = constp.tile([HID, 2 * B], bf16, name="fillsrc")
        logit_sb = constp.tile([HID, CDIM * OUT], f32, name="logit_sb")
        den_sb = constp.tile([HID, CDIM], f32, name="den_sb")

        nc.sync.dma_start(w_sb[:], d_w.ap())
        nc.sync.dma_start(tbl_sb[:], d_tbl.ap())
        nc.sync.dma_start(wfc_sb[:], d_wfc.ap())
        nc.sync.dma_start(bfcw_sb[:], d_bfcw.ap())
        nc.sync.dma_start(cst2[:], d_c0.ap())
        nc.vector.memset(h0_sb[:], 0.0)
        nc.vector.memset(ones_sb[:], 1.0)
        nc.vector.memset(fillsrc[:], 0.0)
        nc.vector.memset(logit_sb[:], 0.0)
        # Pin the sigmoid_and_others table (contains tanh too) before the loop.
        nc.scalar.activation(scr[:], h0_sb[:], AF.Sigmoid)

        cst = [cst2[:, c * B : (c + 1) * B] for c in range(NCH)]

        with (
            tc.tile_pool(name="ohp", bufs=3) as ohp,
            tc.tile_pool(name="gatep", bufs=1, space="PSUM") as gatep,
            tc.tile_pool(name="fcp", bufs=1, space="PSUM") as fcp,
            tc.tile_pool(name="fillp", bufs=1, space="PSUM") as fillp,
            tc.tile_pool(name="ringp", bufs=3) as ringp,
            tc.tile_pool(name="workp", bufs=2) as workp,
        ):
            oh_tiles = [None] * (TT // OG + 1)

            def fetch_oh(chunk):
                ohc = ohp.tile([VK, OG * NCH * B], bf16, tag="oh")
                nc.sync.dma_start(
                    ohc[:],
                    d_oh.ap()[:, chunk * OG * NCH * B : (chunk + 1) * OG * NCH * B],
                )
                oh_tiles[chunk] = ohc

            fetch_oh(0)

            mergesig = os.environ.get("V6_MERGESIG", "1") == "1"

            def alloc_pair(r, c):
                """Allocate chain c's round-r gate PSUM (pair of banks, or one
                2-bank tile in mergesig mode) and prefill from the one-hot
                block (4 matmuls, K=VK, N=256). Quarter order [2g|i|f|o]."""
                if mergesig:
                    psAB = gatep.tile([128, 4 * B], f32, tag=f"psAB{c}")
                    quarters = [psAB[:, j * B : (j + 1) * B] for j in range(4)]
                else:
                    psA = gatep.tile([128, 2 * B], f32, tag=f"psA{c}")
                    psB = gatep.tile([128, 2 * B], f32, tag=f"psB{c}")
                    quarters = [
                        psA[:, 0:B], psA[:, B : 2 * B],
                        psB[:, 0:B], psB[:, B : 2 * B],
                    ]
                oh = oh_tiles[r // OG]
                col = ((r % OG) * NCH + c) * B
                for j, q in enumerate((GG, GI, GF, GO)):
                    nc.tensor.matmul(
                        quarters[j],
                        tbl_sb[:, q * HID : (q + 1) * HID],
                        oh[:, col : col + B],
                        start=(j % 2 == 0),
                        stop=False,
                        skip_group_check=True,
                    )
                if mergesig:
                    return (psAB, quarters)
                return (psA, psB, quarters)

            fill_ps = fillp.tile([128, 2 * B], f32, name="fill_ps")
            # one-time warm-up burst: ~7us of dependency-free back-to-back
            # matmuls locks the HAM clock gate at K=8/8 deterministically
            # (otherwise warm/cold lock-in depends on the free-running HAM
            # window phase at kernel start; a cold run costs ~1us/round).
            # Runs while the input DMAs stream, so it is effectively free.
            for _ in range(32):
                nc.tensor.matmul(
                    fill_ps[:, 0:B], fillsrc[:, 0:HID], fillsrc[:, 0:B],
                    start=True, stop=True, skip_group_check=True,
                )
            cur = [alloc_pair(0, c) for c in range(NCH)]
            nxt = [None] * NCH
            fcw_box = [None]
            pending_fc = [None] * NCH
            h_prev = [h0_sb[:]] * NCH

            def fc_round(t):
                """fc for output step t, every chain with a pending h and a
                valid step. Shared window bank [128, (chain, half, FCW, OUT)]
                f32; window NWIN-1 is partial and evacuated post-loop."""
                w0 = t % FCW
                if w0 == 0:
                    fcw_box[0] = fcp.tile(
                        [HID, 2 * NCH * FCW * OUT], f32, tag="fcw", name="fcw"
                    )
                    nc.tensor.matmul(
                        fcw_box[0][:], ones_sb[:], bfcw_sb[:],
                        start=True, stop=False, skip_group_check=True,
                    )
                fcw = fcw_box[0]
                live = [c for c in range(NCH)
                        if pending_fc[c] is not None and t < SEG3[c]]
                for c in live:
                    for hf in range(2):
                        o0 = (((c * 2) + hf) * FCW + w0) * OUT
                        nc.tensor.matmul(
                            fcw[:, o0 : o0 + OUT],
                            pending_fc[c][:, hf * HID : (hf + 1) * HID],
                            wfc_sb[:],
                            start=False,
                            stop=(hf == 1 and c == live[-1]
                                  and (w0 == FCW - 1 or t == SEG3[0] - 1)),
                            skip_group_check=True,
                        )
                    pending_fc[c] = None
                if w0 == FCW - 1 or t == SEG3[0] - 1:
                    win = t // FCW
                    W = 2 * NCH * FCW * OUT
                    nc.scalar.copy(
                        logit_sb[:, win * W : (win + 1) * W], fcw[:]
                    )

            gpm = os.environ.get("V7_GPM", "0") == "1"
            ROT = ((0, 1, 2), (1, 2, 0), (2, 0, 1))
            for r in range(TT):
                rot = [c for c in ROT[r % NCH] if r < TTC[c]]
                # --- PE: gate matmuls (critical), then fc
                for c in rot:
                    quarters = cur[c][-1]
                    for j, q in enumerate((GG, GI, GF, GO)):
                        nc.tensor.matmul(
                            quarters[j],
                            w_sb[:, q * HID : (q + 1) * HID],
                            h_prev[c], start=False, stop=(j % 2 == 1),
                            skip_group_check=True,
                        )
                if any(p is not None for p in pending_fc):
                    fc_round(r - 1 - BURN)
                # --- ACT sigmoids + DVE cell updates, staggered so queues
                # drain in readiness order
                sgi = [None] * NCH
                sfo = [None] * NCH
                tg = [None] * NCH
                ig = [None] * NCH
                mmb = [None] * NCH
                tcl = [None] * NCH
                hsl = [None] * NCH

                def act_sgi(c):
                    sgi[c] = workp.tile(
                        [HID, 4 * B], bf16, tag=f"sgi{c}", name=f"sgi{c}"
                    )
                    nc.scalar.activation(sgi[c][:], cur[c][0][:], AF.Sigmoid)
                    sfo[c] = sgi[c]

                def dve_head(c):
                    tg[c] = workp.tile([HID, B], bf16, tag=f"tg{c}", name=f"tg{c}")
                    ig[c] = workp.tile([HID, B], bf16, tag=f"ig{c}", name=f"ig{c}")
                    nc.vector.tensor_scalar(
                        tg[c][:], sgi[c][:, 0:B], 2.0, 1.0,
                        op0=ALU.mult, op1=ALU.subtract,
                    )
                    nc.vector.tensor_mul(ig[c][:], tg[c][:], sgi[c][:, B : 2 * B])

                def dve_cell(c):
                    mmb[c] = workp.tile([HID, B], bf16, tag=f"mm{c}", name=f"mm{c}")
                    eng = nc.gpsimd if gpm else nc.vector
                    eng.tensor_mul(
                        mmb[c][:], sfo[c][:, 2 * B : 3 * B], cst[c]
                    )
                    nc.vector.tensor_add(cst[c], mmb[c][:], ig[c][:])

                def act_tanh(c):
                    tcl[c] = workp.tile([HID, B], bf16, tag=f"tcl{c}", name=f"tcl{c}")
                    nc.scalar.activation(tcl[c][:], cst[c], AF.Tanh)

                def dve_h(c):
                    hsl[c] = ringp.tile([HID, B], bf16, tag=f"h{c}", name=f"h{c}")
                    nc.vector.tensor_mul(
                        hsl[c][:], sfo[c][:, 3 * B : 4 * B], tcl[c][:]
                    )

                if len(rot) == NCH:
                    a, b, d = rot
                    act_sgi(a)
                    act_sgi(b)
                    dve_head(a)
                    dve_cell(a)
                    act_tanh(a)
                    act_sgi(d)
                    dve_head(b)
                    dve_h(a)
                    dve_cell(b)
                    act_tanh(b)
                    dve_head(d)
                    dve_h(b)
                    dve_cell(d)
                    act_tanh(d)
                    dve_h(d)
                else:
                    for c in rot:
                        act_sgi(c)
                        dve_head(c)
                        dve_cell(c)
                        act_tanh(c)
                        dve_h(c)

                # --- PE: prefill round r+1 (after the sigmoids that free the
                # recycled banks are emitted — bufs=1 WAR ordering)
                if (r + 1) % OG == 0 and (r + 1) // OG < (TT + OG - 1) // OG:
                    fetch_oh((r + 1) // OG)
                for c in rot:
                    if r + 1 < TTC[c]:
                        nxt[c] = alloc_pair(r + 1, c)
                # fillers ride in the wait-for-h stall before the next gates
                for _ in range(NFILL):
                    nc.tensor.matmul(
                        fill_ps[:, 0:B], w_sb[:, 0:HID], scr[:],
                        start=True, stop=True, skip_group_check=True,
                    )

                for c in rot:
                    h_prev[c] = hsl[c][:]
                    if r >= BURN and (r - BURN) < SEG3[c]:
                        pending_fc[c] = hsl[c][:]
                    if r + 1 < TTC[c]:
                        cur[c] = nxt[c]
            fc_round(SEG3[0] - 1)

        # ---- phase 2: softmax over OUT, windowed; layout is
        # [128, (win, chain, half, w, OUT)] and the host fixes the order. ----
        p3 = logit_sb[:].rearrange("p (c o) -> p c o", o=OUT)
        NW = 4
        q = CDIM // NW
        for k in range(NW):
            lo, hi = k * q, (k + 1) * q
            nc.scalar.activation(
                logit_sb[:, lo * OUT : hi * OUT],
                logit_sb[:, lo * OUT : hi * OUT], AF.Exp,
            )
            nc.vector.reduce_sum(
                den_sb[:, lo:hi], p3[:, lo:hi, :], axis=mybir.AxisListType.X
            )
            nc.vector.reciprocal(den_sb[:, lo:hi], den_sb[:, lo:hi])
            rec_b = den_sb[:, lo:hi].unsqueeze(2).broadcast_to([HID, q, OUT])
            nc.vector.tensor_mul(p3[:, lo:hi, :], p3[:, lo:hi, :], rec_b)
            nc.sync.dma_start(d_out.ap()[:, lo:hi, :], p3[:, lo:hi, :])

    if not for_sim:
        _split_overloaded_waits(nc, mybir)
    _BUILD_CACHE[key] = nc
    return nc


def _host_prep(inputs, c0, W_ih, W_hh, b_ih, b_hh, W_fc, b_fc, emb):
    import ml_dtypes

    bf16 = ml_dtypes.bfloat16
    inputs = np.asarray(inputs)
    table_f = np.zeros((VK, 4 * HID), np.float32)
    table_f[:VOCAB] = emb @ W_ih.T + (b_ih + b_hh)
    table_f[VOCAB, GI * HID : (GI + 1) * HID] = -30.0
    table_f[VOCAB, GF * HID : (GF + 1) * HID] = 30.0
    table_f[VOCAB, GO * HID : (GO + 1) * HID] = -30.0
    w_f = W_hh.T.copy()
    table_f[:, GG * HID : (GG + 1) * HID] *= 2.0
    w_f[:, GG * HID : (GG + 1) * HID] *= 2.0
    table = table_f.astype(bf16)
    w = np.ascontiguousarray(w_f.astype(bf16))
    wfc = np.ascontiguousarray(W_fc.T.astype(bf16))
    bfcw = np.ascontiguousarray(
        np.tile(b_fc.astype(bf16), 2 * NCH * FCW).reshape(1, 2 * NCH * FCW * OUT)
    )
    c0T = np.ascontiguousarray(c0[0].T.astype(bf16))
    TTP = ((TT + OG - 1) // OG) * OG
    in_maps = []
    for core in range(NCORES):
        tok_all = np.zeros((TTP, NCH, B), np.int64)
        for c in range(NCH):
            st = core * SEG + CB[c]
            if core == 0 and c == 0:
                tok = np.concatenate(
                    [np.full((B, BURN), VOCAB, np.int64),
                     inputs[:, : SEG3[c]]], axis=1,
                )
            else:
                tok = inputs[:, st - BURN : st + SEG3[c]]
            tok_all[: TTC[c], c, :] = tok.T
        oh = np.zeros((VK, TTP * NCH * B), dtype=bf16)
        cols = np.arange(TTP * NCH * B)
        oh[tok_all.reshape(-1), cols] = 1.0
        cc = np.zeros((HID, NCH * B), dtype=bf16)
        if core == 0:
            cc[:, 0:B] = c0T
        in_maps.append(
            {
                "onehot": oh,
                "c0T": cc,
                "w": w,
                "tbl": table,
                "wfc": wfc,
                "bfcw": bfcw,
            }
        )
    return in_maps


def _run(inputs, c0, W_ih, W_hh, b_ih, b_hh, W_fc, b_fc, emb, trace=False):
    from concourse.bass_utils import run_bass_kernel_spmd

    nc = _build_nc()
    in_maps = _host_prep(inputs, c0, W_ih, W_hh, b_ih, b_hh, W_fc, b_fc, emb)
    res = run_bass_kernel_spmd(
        nc, in_maps, core_ids=list(range(NCORES)), trace=trace
    )
    segs = []
    for core in range(NCORES):
        arr = res.results[core]["out"]  # [128, NWIN*NCH*2*FCW, 6]
        arr = arr.reshape(HID, NWIN, NCH, 2, FCW, OUT)
        out_c = np.empty((B, SEG, OUT), np.float32)
        for c in range(NCH):
            for win in range(NWIN):
                wv = min(FCW, SEG3[c] - win * FCW)
                if wv <= 0:
                    break
                blk = arr[:, win, c, :, :wv, :]        # [128, 2, wv, 6]
                blk = blk.transpose(1, 0, 2, 3).reshape(B, wv, OUT)
                t0c = CB[c] + win * FCW
                out_c[:, t0c : t0c + wv] = blk
        segs.append(out_c)
    out = np.concatenate(segs, axis=1)
    return out, res


def kernel(inputs, c0, W_ih, W_hh, b_ih, b_hh, W_fc, b_fc, emb):
    out, _ = _run(
        np.asarray(inputs), np.asarray(c0), np.asarray(W_ih), np.asarray(W_hh),
        np.asarray(b_ih), np.asarray(b_hh), np.asarray(W_fc), np.asarray(b_fc),
        np.asarray(emb),
    )
    return out
